# revision 37
# baseline (speedup 1.0000x reference)
"""Multi-head attention (B=4, S=2048, D=1024, H=16) on 8 NeuronCores.

Sharding: core (b, hg) with b = cid//2, hg = cid%2 computes the partial
output contribution of head-group hg (8 heads) of batch b:
    part = softmax((x_q Wq_hg^T + bq_hg)(x_k Wq_hg^T + bq_hg)^T / 8) (x_v ...) Wo[:, hg]^T
Host sums the two partials per batch and adds bo.

Kernel internals (per core):
  phase 1 (v, k, then q interleaved below): inputs loaded bf16 and
           transposed on the PE (8x 128x128 transposes per chunk,
           PSUM->SBUF copies split ACT/DVE) -- keeps the DMA engines free
           for the loads; transpose of group g+1 pipelined ahead of the
           projection of group g. In-proj matmuls produce qpT/kpT
           [512, 2048] f32r (dims on partitions) and vp natural
           [2048, 512] with a ones column per head (VPA [2048, 8*65]) so
           the PV matmul also emits the softmax denominator.
  phase 2: loop q-blocks (512 q) outer, head pairs inner; each block's
           q-projection and the next block's loads/DMA-transposes are
           interleaved between attention blocks. Per (pair, kc): scoresT
           [keys,q] via two K=64 matmuls issued back-to-back (opposite PE
           row halves -> row-tiled concurrency on HW), software-pipelined
           one kc ahead of PV; exp on ACT (7/12, exact) or DVE (5/12,
           Schraudolph bf16-bit trick); PV uses att chunks [keys,128q] as
           the stationary operand (full 128x128 array, FWL-eligible, N=65
           moving V+ones) accumulating ctx natural [q, 65] in PSUM (one
           accumulation group per 2KB zero region). Normalize = DVE
           reciprocal of the denominator column + per-partition scalar
           multiply into conc bf16 [q, 512].
  phase 3: per q-block (overlapped with the next block's attention):
           DMA-transpose conc -> concT [dim, seq] (PE transpose for the
           last block), out-proj matmuls, PSUM->SBUF copy, DMA to DRAM.
"""

import math

import numpy as np

from concourse import bacc
import concourse.mybir as mybir
import concourse.tile as tile
from concourse.masks import make_identity

f32 = mybir.dt.float32
f32r = mybir.dt.float32r
bf16 = mybir.dt.bfloat16
AF = mybir.ActivationFunctionType
i16 = mybir.dt.int16
# Schraudolph exp for bf16 bit pattern: bf16_bits = round(2^7*(s*0.125/ln2 + 127 - c))
SCHRAUD_A = 128.0 * 0.125 / math.log(2.0)
SCHRAUD_B = 128.0 * (127.0 - 0.0450466) + 0.5

P = 128
S = 2048           # sequence length
D = 1024           # model dim
DH = 512           # head-group dim (8 heads x 64)
HD = 64            # head dim
NH = 8             # heads per core
SC = S // P        # 16 seq chunks
KC = D // P        # 8 contraction chunks (model dim)
MC = DH // P       # 4 out-dim chunks
QB = 512           # q-block size for phase 2/3
NQB = S // QB      # 4 q blocks
NQC = QB // P      # 4 128-q chunks per block


def build_kernel():
    nc = bacc.Bacc(None, target_bir_lowering=False)
    xq = nc.dram_tensor("xq", [S, D], f32, kind="ExternalInput")
    xk = nc.dram_tensor("xk", [S, D], f32, kind="ExternalInput")
    xv = nc.dram_tensor("xv", [S, D], f32, kind="ExternalInput")
    wqt = nc.dram_tensor("wqt", [D, DH], f32, kind="ExternalInput")   # Wq_hg.T
    bq = nc.dram_tensor("bq", [DH], f32, kind="ExternalInput")
    wot = nc.dram_tensor("wot", [DH, D], f32, kind="ExternalInput")   # Wo[:, hg].T
    out = nc.dram_tensor("out", [S, D], f32, kind="ExternalOutput")

    with tile.TileContext(nc) as tc:
        with tc.tile_pool(name="singles", bufs=1) as singles:
            # ---- constants / weights ----
            # keep the Pool (SWDGE) queue free for the xn input loads:
            # constants go through SP/ACT/DVE HWDGE queues
            WQT = singles.tile([P, KC, DH], bf16)  # loaded after first x loads
            BQT = singles.tile([P, MC], f32)
            nc.sync.dma_start(BQT, bq[:].rearrange("(mc p) -> p mc", p=P))
            BQB = singles.tile([P, DH], f32)
            nc.scalar.dma_start(BQB, bq[:].partition_broadcast(P))
            WOT = singles.tile([P, MC, D], bf16)  # loaded later (phase 3 only)
            IDN = singles.tile([P, P], bf16)
            make_identity(nc, IDN)

            # ---- persistent activations ----
            QPT = singles.tile([P, MC, S], f32r)    # qpT: [dim, seq]
            KPT = singles.tile([P, MC, S], f32r)
            CONCT = singles.tile([P, MC, S], bf16)
            VPA = singles.tile([P, SC, NH * (HD + 1)], bf16)  # vp + ones cols
            vones = (
                VPA[:, :, :]
                .rearrange("p sc (h c) -> p sc h c", h=NH)[:, :, :, HD:HD + 1]
            )
            nc.gpsimd.memset(vones, 1.0)

            # =========== phase 1: transpose inputs + projections ===========
            with (
                tc.tile_pool(name="xn", bufs=4) as xn_pool,
                tc.tile_pool(name="xt", bufs=2) as xt_pool,
                tc.tile_pool(name="att", bufs=6) as at_pool,
                tc.tile_pool(name="cnc", bufs=2) as cn_pool,
                tc.tile_pool(name="rcp", bufs=8) as rc_pool,
                tc.tile_pool(name="osb", bufs=4) as ob_pool,
                # PSUM: "big" shared by input-transpose + scores (4 banks),
                # "acc" shared by in-proj + out-proj accumulators (2 banks),
                # "cps" ctx accumulators (2 banks) -> 8 banks total
                tc.tile_pool(name="big", bufs=4, space="PSUM") as sp_pool,
                tc.tile_pool(name="acc", bufs=2, space="PSUM") as ppool,
                tc.tile_pool(name="cps", bufs=2, space="PSUM") as cp_pool,
            ):
                expi = 0

                def load_chunk(xin, sc):
                    xn = xn_pool.tile([P, D], bf16, tag="xn", name="xn")
                    nc.gpsimd.dma_start(xn, xin[sc * P:(sc + 1) * P, :])
                    return xn

                def transp_group(t, xin, g, preloaded=None, force_pe=False):
                    """Load + transpose one 512-seq group into an xt tile."""
                    xt = xt_pool.tile([P, KC, 512], bf16, tag="xt", name="xt")
                    for m in range(4):
                        sc = g * 4 + m
                        xn = (
                            preloaded[m] if preloaded is not None
                            else load_chunk(xin, sc)
                        )
                        dst = xt[:, :, m * P:(m + 1) * P]
                        if t == 2 and not force_pe:
                            # q groups run interleaved with attention: the
                            # DMA engines are idle there, PE/ACT/DVE are not
                            nc.sync.dma_start(dst, xn, transpose=True)
                            continue
                        # PE transpose + copy (PSUM -> SBUF)
                        tp = sp_pool.tile([P, D], bf16, tag="big", name="tp")
                        for j in range(KC):
                            nc.tensor.transpose(
                                tp[:, j * P:(j + 1) * P],
                                xn[:, j * P:(j + 1) * P],
                                IDN,
                            )
                        src = tp.rearrange("p (k s) -> p k s", k=KC)
                        if m % 2 == 0:
                            nc.vector.tensor_copy(dst, src)
                        else:
                            nc.scalar.copy(dst, src)
                    return xt

                def proj_group(t, g, xt):
                    if t == 0:            # xv -> VPA (natural layout)
                        for m in range(4):
                            sc = g * 4 + m
                            ps = ppool.tile([P, 512], f32, tag="acc", name="pp")
                            for kc in range(KC):
                                nc.tensor.matmul(
                                    ps,
                                    xt[:, kc, m * P:(m + 1) * P],
                                    WQT[:, kc, :],
                                    start=(kc == 0),
                                    stop=(kc == KC - 1),
                                )
                            nc.vector.tensor_add(
                                VPA[:, sc, :]
                                .rearrange("p (h c) -> p h c", h=NH)[:, :, 0:HD],
                                ps.rearrange("p (h c) -> p h c", h=NH),
                                BQB.rearrange("p (h c) -> p h c", h=NH),
                            )
                    else:                 # xk/xq -> KPT/QPT ([dim, seq])
                        dstt = KPT if t == 1 else QPT
                        for mc in range(MC):
                            ps = ppool.tile([P, 512], f32, tag="acc", name="pp")
                            for kc in range(KC):
                                nc.tensor.matmul(
                                    ps,
                                    WQT[:, kc, mc * P:(mc + 1) * P],
                                    xt[:, kc, :],
                                    start=(kc == 0),
                                    stop=(kc == KC - 1),
                                )
                            nc.scalar.activation(
                                dstt[:, mc, g * 512:(g + 1) * 512],
                                ps,
                                AF.Identity,
                                bias=BQT[:, mc:mc + 1],
                                scale=1.0,
                            )

                # v and k fully first (phase 2 needs all keys/values).
                # First group's loads go ahead of the WQT load on the Pool
                # queue so the PE can start transposing immediately. The
                # transpose of group g+1 is issued before the projection of
                # group g so the PSUM->SBUF copies hide under the matmuls.
                xns0 = [load_chunk(xv, m) for m in range(4)]
                # split the weight load so the first projection matmuls can
                # start as soon as the first half lands
                wqt_r = wqt[:].rearrange("(kc p) m -> p kc m", p=P)
                nc.gpsimd.dma_start(WQT[:, 0:4, :], wqt_r[:, 0:4, :])
                nc.gpsimd.dma_start(WQT[:, 4:KC, :], wqt_r[:, 4:KC, :])
                vk = [(0, xv, g) for g in range(4)] + [(1, xk, g) for g in range(4)]
                vk += [(2, xq, 0)]
                xt_cur = transp_group(0, xv, 0, preloaded=xns0)
                for i, (t, xin, g) in enumerate(vk):
                    if i + 1 < len(vk):
                        tn, xinn, gn = vk[i + 1]
                        # q group 0 is not covered by an attention block:
                        # PE transposes avoid the DMA-transpose latency chain
                        xt_nxt = transp_group(tn, xinn, gn, force_pe=(tn == 2))
                    proj_group(t, g, xt_cur)
                    xt_cur = xt_nxt
                nc.gpsimd.dma_start(
                    WOT, wot[:].rearrange("(mc p) n -> p mc n", p=P)
                )

                for qb in range(NQB):
                    qsl = slice(qb * QB, (qb + 1) * QB)
                    conc = [
                        cn_pool.tile([P, DH], bf16, tag=f"c{qc}", name=f"conc{qc}")
                        for qc in range(NQC)
                    ]
                    for hp in range(4):       # head pairs
                        heads = ((2 * hp, 0), (2 * hp + 1, HD))
                        cps = {}
                        for h, po in heads:
                            cps[h] = cp_pool.tile(
                                [P, NQC, HD + 1], f32, tag="cps", name=f"cps{h}"
                            )

                        def issue_scores(po, kc):
                            sp = sp_pool.tile([P, QB], f32, tag="big", name="sps")
                            nc.tensor.matmul(
                                sp,
                                KPT[po:po + HD, hp, kc * P:(kc + 1) * P],
                                QPT[po:po + HD, hp, qsl],
                                start=True,
                                stop=True,
                            )
                            return sp

                        # software-pipeline scores one kc ahead of PV
                        cur = {po: issue_scores(po, 0) for _, po in heads}
                        for kc in range(SC):
                            nxt = (
                                {po: issue_scores(po, kc + 1) for _, po in heads}
                                if kc + 1 < SC
                                else None
                            )
                            for h, po in heads:
                                att = at_pool.tile([P, QB], bf16, tag="att")
                                if (expi * 5) % 12 < 5:
                                    nc.vector.tensor_scalar(
                                        att.bitcast(i16), cur[po],
                                        SCHRAUD_A, SCHRAUD_B,
                                        op0=mybir.AluOpType.mult,
                                        op1=mybir.AluOpType.add,
                                    )
                                else:
                                    nc.scalar.activation(
                                        att, cur[po], AF.Exp, scale=0.125
                                    )
                                expi += 1
                                for qc in range(NQC):
                                    # one accumulation group per 2KB zero
                                    # region: start/stop only on the very
                                    # first/last matmul touching the tile
                                    nc.tensor.matmul(
                                        cps[h][:, qc, :],
                                        att[:, qc * P:(qc + 1) * P],
                                        VPA[:, kc, h * (HD + 1):(h + 1) * (HD + 1)],
                                        start=(kc == 0 and qc == 0),
                                        stop=(kc == SC - 1 and qc == NQC - 1),
                                    )
                            cur = nxt
                        # normalize: per-partition reciprocal + scale
                        for h, po in heads:
                            rc = rc_pool.tile([P, NQC], f32, tag="rc")
                            nc.vector.reciprocal(rc, cps[h][:, :, HD])
                            for qc in range(NQC):
                                nc.vector.tensor_scalar(
                                    conc[qc][:, h * HD:(h + 1) * HD],
                                    cps[h][:, qc, 0:HD],
                                    rc[:, qc:qc + 1],
                                    None,
                                    op0=mybir.AluOpType.mult,
                                )
                                if hp == 3 and h == heads[1][0]:
                                    sc = qb * NQC + qc
                                    if qb < NQB - 1:
                                        nc.sync.dma_start(
                                            CONCT[:, :, sc * P:(sc + 1) * P],
                                            conc[qc],
                                            transpose=True,
                                        )
                                    else:
                                        # last block: no attention left to
                                        # hide the DMA latency -> PE transpose
                                        tp = sp_pool.tile(
                                            [P, DH], bf16, tag="big", name="tpc"
                                        )
                                        for j in range(MC):
                                            nc.tensor.transpose(
                                                tp[:, j * P:(j + 1) * P],
                                                conc[qc][:, j * P:(j + 1) * P],
                                                IDN,
                                            )
                                        nc.vector.tensor_copy(
                                            CONCT[:, :, sc * P:(sc + 1) * P],
                                            tp.rearrange(
                                                "p (k s) -> p k s", k=MC
                                            ),
                                        )
                    # prefetch next q block (its DMA transposes must beat
                    # phase 3's out-DMAs onto the SP queue)
                    if qb + 1 < NQB:
                        xt_q = transp_group(2, xq, qb + 1)
                        proj_group(2, qb + 1, xt_q)
                    # ---- phase 3 for this q block ----
                    for qc in range(NQC):
                        sc = qb * NQC + qc
                        for n in range(D // 512):
                            ps = ppool.tile([P, 512], f32, tag="acc", name="ops")
                            for mc in range(MC):
                                nc.tensor.matmul(
                                    ps,
                                    CONCT[:, mc, sc * P:(sc + 1) * P],
                                    WOT[:, mc, n * 512:(n + 1) * 512],
                                    start=(mc == 0),
                                    stop=(mc == MC - 1),
                                )
                            ob = ob_pool.tile([P, 512], f32, tag="ob")
                            nc.vector.tensor_copy(ob, ps)
                            nc.sync.dma_start(
                                out[sc * P:(sc + 1) * P, n * 512:(n + 1) * 512],
                                ob,
                            )
    nc.finalize()
    return nc


_NC = None


def _get_nc():
    global _NC
    if _NC is None:
        _NC = build_kernel()
    return _NC


def kernel(q, k, v, Wq, bq, Wo, bo, _trace=False):
    from concourse.bass_utils import run_bass_kernel_spmd

    q = np.asarray(q, dtype=np.float32)
    k = np.asarray(k, dtype=np.float32)
    v = np.asarray(v, dtype=np.float32)
    Wq = np.asarray(Wq, dtype=np.float32)
    bq = np.asarray(bq, dtype=np.float32)
    Wo = np.asarray(Wo, dtype=np.float32)
    bo = np.asarray(bo, dtype=np.float32)

    nc = _get_nc()
    B = q.shape[0]
    in_maps = []
    for cid in range(8):
        b, hg = cid // 2, cid % 2
        sl = slice(hg * DH, (hg + 1) * DH)
        in_maps.append({
            "xq": np.ascontiguousarray(q[b]),
            "xk": np.ascontiguousarray(k[b]),
            "xv": np.ascontiguousarray(v[b]),
            "wqt": np.ascontiguousarray(Wq[sl, :].T),
            "bq": np.ascontiguousarray(bq[sl]),
            "wot": np.ascontiguousarray(Wo[:, sl].T),
        })
    try:
        res = run_bass_kernel_spmd(
            nc, in_maps, core_ids=list(range(8)), trace=_trace
        )
    except ModuleNotFoundError:
        # NTFF profiling hook unavailable (axon client without antenv
        # profiling support) -- rerun without tracing
        res = run_bass_kernel_spmd(
            nc, in_maps, core_ids=list(range(8)), trace=False
        )
    parts = [r["out"] for r in res.results]
    outv = np.stack([parts[2 * b] + parts[2 * b + 1] for b in range(B)])
    outv = outv + bo[None, None, :]
    if _trace:
        kernel.last_result = res
    return outv[None].astype(np.float32)


# revision 38
# speedup vs baseline: 1.0066x; 1.0066x over previous
"""Multi-head attention (B=4, S=2048, D=1024, H=16) on 8 NeuronCores.

Sharding: core (b, hg) with b = cid//2, hg = cid%2 computes the partial
output contribution of head-group hg (8 heads) of batch b:
    part = softmax((x_q Wq_hg^T + bq_hg)(x_k Wq_hg^T + bq_hg)^T / 8) (x_v ...) Wo[:, hg]^T
Host sums the two partials per batch and adds bo.

Kernel internals (per core):
  phase 1 (v, k, then q interleaved below): inputs loaded bf16 and
           transposed on the PE (8x 128x128 transposes per chunk,
           PSUM->SBUF copies split ACT/DVE) -- keeps the DMA engines free
           for the loads; transpose of group g+1 pipelined ahead of the
           projection of group g. In-proj matmuls produce qpT/kpT
           [512, 2048] f32r (dims on partitions) and vp natural
           [2048, 512] with a ones column per head (VPA [2048, 8*65]) so
           the PV matmul also emits the softmax denominator.
  phase 2: loop q-blocks (512 q) outer, head pairs inner; each block's
           q-projection and the next block's loads/DMA-transposes are
           interleaved between attention blocks. Per (pair, kc): scoresT
           [keys,q] via two K=64 matmuls issued back-to-back (opposite PE
           row halves -> row-tiled concurrency on HW), software-pipelined
           one kc ahead of PV; exp on ACT (7/12, exact) or DVE (5/12,
           Schraudolph bf16-bit trick); PV uses att chunks [keys,128q] as
           the stationary operand (full 128x128 array, FWL-eligible, N=65
           moving V+ones) accumulating ctx natural [q, 65] in PSUM (one
           accumulation group per 2KB zero region). Normalize = DVE
           reciprocal of the denominator column + per-partition scalar
           multiply into conc bf16 [q, 512].
  phase 3: per q-block (overlapped with the next block's attention):
           DMA-transpose conc -> concT [dim, seq] (PE transpose for the
           last block), out-proj matmuls, PSUM->SBUF copy, DMA to DRAM.
"""

import math

import numpy as np

from concourse import bacc
import concourse.mybir as mybir
import concourse.tile as tile
from concourse.masks import make_identity

f32 = mybir.dt.float32
f32r = mybir.dt.float32r
bf16 = mybir.dt.bfloat16
AF = mybir.ActivationFunctionType
i16 = mybir.dt.int16
# Schraudolph exp for bf16 bit pattern: bf16_bits = round(2^7*(s*0.125/ln2 + 127 - c))
SCHRAUD_A = 128.0 * 0.125 / math.log(2.0)
SCHRAUD_B = 128.0 * (127.0 - 0.0450466) + 0.5

P = 128
S = 2048           # sequence length
D = 1024           # model dim
DH = 512           # head-group dim (8 heads x 64)
HD = 64            # head dim
NH = 8             # heads per core
SC = S // P        # 16 seq chunks
KC = D // P        # 8 contraction chunks (model dim)
MC = DH // P       # 4 out-dim chunks
QB = 512           # q-block size for phase 2/3
NQB = S // QB      # 4 q blocks
NQC = QB // P      # 4 128-q chunks per block


def build_kernel():
    nc = bacc.Bacc(None, target_bir_lowering=False)
    xq = nc.dram_tensor("xq", [S, D], f32, kind="ExternalInput")
    xk = nc.dram_tensor("xk", [S, D], f32, kind="ExternalInput")
    xv = nc.dram_tensor("xv", [S, D], f32, kind="ExternalInput")
    wqt = nc.dram_tensor("wqt", [D, DH], f32, kind="ExternalInput")   # Wq_hg.T
    bq = nc.dram_tensor("bq", [DH], f32, kind="ExternalInput")
    wot = nc.dram_tensor("wot", [DH, D], f32, kind="ExternalInput")   # Wo[:, hg].T
    out = nc.dram_tensor("out", [S, D], f32, kind="ExternalOutput")

    with tile.TileContext(nc) as tc:
        with tc.tile_pool(name="singles", bufs=1) as singles:
            # ---- constants / weights ----
            # keep the Pool (SWDGE) queue free for the xn input loads:
            # constants go through SP/ACT/DVE HWDGE queues
            WQT = singles.tile([P, KC, DH], bf16)  # loaded after first x loads
            BQT = singles.tile([P, MC], f32)
            nc.sync.dma_start(BQT, bq[:].rearrange("(mc p) -> p mc", p=P))
            BQB = singles.tile([P, DH], f32)
            nc.scalar.dma_start(BQB, bq[:].partition_broadcast(P))
            WOT = singles.tile([P, MC, D], bf16)  # loaded later (phase 3 only)
            IDN = singles.tile([P, P], bf16)
            make_identity(nc, IDN)

            # ---- persistent activations ----
            QPT = singles.tile([P, MC, S], f32r)    # qpT: [dim, seq]
            KPT = singles.tile([P, MC, S], f32r)
            CONCT = singles.tile([P, MC, S], bf16)
            VPA = singles.tile([P, SC, NH * (HD + 1)], bf16)  # vp + ones cols
            vones = (
                VPA[:, :, :]
                .rearrange("p sc (h c) -> p sc h c", h=NH)[:, :, :, HD:HD + 1]
            )
            nc.gpsimd.memset(vones, 1.0)

            # =========== phase 1: transpose inputs + projections ===========
            with (
                tc.tile_pool(name="xn", bufs=4) as xn_pool,
                tc.tile_pool(name="xt", bufs=3) as xt_pool,
                tc.tile_pool(name="att", bufs=6) as at_pool,
                tc.tile_pool(name="cnc", bufs=2) as cn_pool,
                tc.tile_pool(name="rcp", bufs=8) as rc_pool,
                tc.tile_pool(name="osb", bufs=4) as ob_pool,
                # PSUM: "big" shared by input-transpose + scores (4 banks),
                # "acc" shared by in-proj + out-proj accumulators (2 banks),
                # "cps" ctx accumulators (2 banks) -> 8 banks total
                tc.tile_pool(name="big", bufs=4, space="PSUM") as sp_pool,
                tc.tile_pool(name="acc", bufs=2, space="PSUM") as ppool,
                tc.tile_pool(name="cps", bufs=2, space="PSUM") as cp_pool,
            ):
                expi = 0

                def load_chunk(xin, sc):
                    xn = xn_pool.tile([P, D], bf16, tag="xn", name="xn")
                    nc.gpsimd.dma_start(xn, xin[sc * P:(sc + 1) * P, :])
                    return xn

                def transp_group(t, xin, g, preloaded=None, force_pe=False):
                    """Load + transpose one 512-seq group into an xt tile."""
                    xt = xt_pool.tile([P, KC, 512], bf16, tag="xt", name="xt")
                    for m in range(4):
                        sc = g * 4 + m
                        xn = (
                            preloaded[m] if preloaded is not None
                            else load_chunk(xin, sc)
                        )
                        dst = xt[:, :, m * P:(m + 1) * P]
                        if t == 2 and not force_pe:
                            # q groups run interleaved with attention: the
                            # DMA engines are idle there, PE/ACT/DVE are not
                            nc.sync.dma_start(dst, xn, transpose=True)
                            continue
                        # PE transpose + copy (PSUM -> SBUF)
                        tp = sp_pool.tile([P, D], bf16, tag="big", name="tp")
                        for j in range(KC):
                            nc.tensor.transpose(
                                tp[:, j * P:(j + 1) * P],
                                xn[:, j * P:(j + 1) * P],
                                IDN,
                            )
                        src = tp.rearrange("p (k s) -> p k s", k=KC)
                        if m % 2 == 0:
                            nc.vector.tensor_copy(dst, src)
                        else:
                            nc.scalar.copy(dst, src)
                    return xt

                def proj_group(t, g, xt):
                    if t == 0:            # xv -> VPA (natural layout)
                        for m in range(4):
                            sc = g * 4 + m
                            ps = ppool.tile([P, 512], f32, tag="acc", name="pp")
                            for kc in range(KC):
                                nc.tensor.matmul(
                                    ps,
                                    xt[:, kc, m * P:(m + 1) * P],
                                    WQT[:, kc, :],
                                    start=(kc == 0),
                                    stop=(kc == KC - 1),
                                )
                            nc.vector.tensor_add(
                                VPA[:, sc, :]
                                .rearrange("p (h c) -> p h c", h=NH)[:, :, 0:HD],
                                ps.rearrange("p (h c) -> p h c", h=NH),
                                BQB.rearrange("p (h c) -> p h c", h=NH),
                            )
                    else:                 # xk/xq -> KPT/QPT ([dim, seq])
                        dstt = KPT if t == 1 else QPT
                        for mc in range(MC):
                            ps = ppool.tile([P, 512], f32, tag="acc", name="pp")
                            for kc in range(KC):
                                nc.tensor.matmul(
                                    ps,
                                    WQT[:, kc, mc * P:(mc + 1) * P],
                                    xt[:, kc, :],
                                    start=(kc == 0),
                                    stop=(kc == KC - 1),
                                )
                            nc.scalar.activation(
                                dstt[:, mc, g * 512:(g + 1) * 512],
                                ps,
                                AF.Identity,
                                bias=BQT[:, mc:mc + 1],
                                scale=1.0,
                            )

                # v and k fully first (phase 2 needs all keys/values).
                # First group's loads go ahead of the WQT load on the Pool
                # queue so the PE can start transposing immediately. The
                # transpose of group g+1 is issued before the projection of
                # group g so the PSUM->SBUF copies hide under the matmuls.
                xns0 = [load_chunk(xv, m) for m in range(4)]
                # split the weight load so the first projection matmuls can
                # start as soon as the first half lands
                wqt_r = wqt[:].rearrange("(kc p) m -> p kc m", p=P)
                nc.gpsimd.dma_start(WQT[:, 0:4, :], wqt_r[:, 0:4, :])
                nc.gpsimd.dma_start(WQT[:, 4:KC, :], wqt_r[:, 4:KC, :])
                vk = [(0, xv, g) for g in range(4)] + [(1, xk, g) for g in range(4)]
                vk += [(2, xq, 0)]
                xt_cur = transp_group(0, xv, 0, preloaded=xns0)
                for i, (t, xin, g) in enumerate(vk):
                    if i + 1 < len(vk):
                        tn, xinn, gn = vk[i + 1]
                        # q group 0 is not covered by an attention block:
                        # PE transposes avoid the DMA-transpose latency chain
                        xt_nxt = transp_group(tn, xinn, gn, force_pe=(tn == 2))
                    proj_group(t, g, xt_cur)
                    xt_cur = xt_nxt
                nc.gpsimd.dma_start(
                    WOT, wot[:].rearrange("(mc p) n -> p mc n", p=P)
                )

                for qb in range(NQB):
                    qsl = slice(qb * QB, (qb + 1) * QB)
                    conc = [
                        cn_pool.tile([P, DH], bf16, tag=f"c{qc}", name=f"conc{qc}")
                        for qc in range(NQC)
                    ]
                    for hp in range(4):       # head pairs
                        heads = ((2 * hp, 0), (2 * hp + 1, HD))
                        cps = {}
                        for h, po in heads:
                            cps[h] = cp_pool.tile(
                                [P, NQC, HD + 1], f32, tag="cps", name=f"cps{h}"
                            )

                        def issue_scores(po, kc):
                            sp = sp_pool.tile([P, QB], f32, tag="big", name="sps")
                            nc.tensor.matmul(
                                sp,
                                KPT[po:po + HD, hp, kc * P:(kc + 1) * P],
                                QPT[po:po + HD, hp, qsl],
                                start=True,
                                stop=True,
                            )
                            return sp

                        # software-pipeline scores one kc ahead of PV
                        cur = {po: issue_scores(po, 0) for _, po in heads}
                        for kc in range(SC):
                            nxt = (
                                {po: issue_scores(po, kc + 1) for _, po in heads}
                                if kc + 1 < SC
                                else None
                            )
                            for h, po in heads:
                                att = at_pool.tile([P, QB], bf16, tag="att")
                                if (expi * 5) % 12 < 5:
                                    nc.vector.tensor_scalar(
                                        att.bitcast(i16), cur[po],
                                        SCHRAUD_A, SCHRAUD_B,
                                        op0=mybir.AluOpType.mult,
                                        op1=mybir.AluOpType.add,
                                    )
                                else:
                                    nc.scalar.activation(
                                        att, cur[po], AF.Exp, scale=0.125
                                    )
                                expi += 1
                                for qc in range(NQC):
                                    # one accumulation group per 2KB zero
                                    # region: start/stop only on the very
                                    # first/last matmul touching the tile
                                    nc.tensor.matmul(
                                        cps[h][:, qc, :],
                                        att[:, qc * P:(qc + 1) * P],
                                        VPA[:, kc, h * (HD + 1):(h + 1) * (HD + 1)],
                                        start=(kc == 0 and qc == 0),
                                        stop=(kc == SC - 1 and qc == NQC - 1),
                                    )
                            cur = nxt
                        # normalize: per-partition reciprocal + scale
                        for h, po in heads:
                            rc = rc_pool.tile([P, NQC], f32, tag="rc")
                            nc.vector.reciprocal(rc, cps[h][:, :, HD])
                            for qc in range(NQC):
                                nc.vector.tensor_scalar(
                                    conc[qc][:, h * HD:(h + 1) * HD],
                                    cps[h][:, qc, 0:HD],
                                    rc[:, qc:qc + 1],
                                    None,
                                    op0=mybir.AluOpType.mult,
                                )
                                if hp == 3 and h == heads[1][0]:
                                    sc = qb * NQC + qc
                                    if qb < NQB - 1:
                                        nc.sync.dma_start(
                                            CONCT[:, :, sc * P:(sc + 1) * P],
                                            conc[qc],
                                            transpose=True,
                                        )
                                    else:
                                        # last block: no attention left to
                                        # hide the DMA latency -> PE transpose
                                        tp = sp_pool.tile(
                                            [P, DH], bf16, tag="big", name="tpc"
                                        )
                                        for j in range(MC):
                                            nc.tensor.transpose(
                                                tp[:, j * P:(j + 1) * P],
                                                conc[qc][:, j * P:(j + 1) * P],
                                                IDN,
                                            )
                                        nc.vector.tensor_copy(
                                            CONCT[:, :, sc * P:(sc + 1) * P],
                                            tp.rearrange(
                                                "p (k s) -> p k s", k=MC
                                            ),
                                        )
                    # prefetch next q block (its DMA transposes must beat
                    # phase 3's out-DMAs onto the SP queue)
                    if qb + 1 < NQB:
                        xt_q = transp_group(2, xq, qb + 1)
                        proj_group(2, qb + 1, xt_q)
                    # ---- phase 3 for this q block ----
                    for qc in range(NQC):
                        sc = qb * NQC + qc
                        for n in range(D // 512):
                            ps = ppool.tile([P, 512], f32, tag="acc", name="ops")
                            for mc in range(MC):
                                nc.tensor.matmul(
                                    ps,
                                    CONCT[:, mc, sc * P:(sc + 1) * P],
                                    WOT[:, mc, n * 512:(n + 1) * 512],
                                    start=(mc == 0),
                                    stop=(mc == MC - 1),
                                )
                            ob = ob_pool.tile([P, 512], f32, tag="ob")
                            nc.vector.tensor_copy(ob, ps)
                            nc.sync.dma_start(
                                out[sc * P:(sc + 1) * P, n * 512:(n + 1) * 512],
                                ob,
                            )
    nc.finalize()
    return nc


_NC = None


def _get_nc():
    global _NC
    if _NC is None:
        _NC = build_kernel()
    return _NC


def kernel(q, k, v, Wq, bq, Wo, bo, _trace=False):
    from concourse.bass_utils import run_bass_kernel_spmd

    q = np.asarray(q, dtype=np.float32)
    k = np.asarray(k, dtype=np.float32)
    v = np.asarray(v, dtype=np.float32)
    Wq = np.asarray(Wq, dtype=np.float32)
    bq = np.asarray(bq, dtype=np.float32)
    Wo = np.asarray(Wo, dtype=np.float32)
    bo = np.asarray(bo, dtype=np.float32)

    nc = _get_nc()
    B = q.shape[0]
    in_maps = []
    for cid in range(8):
        b, hg = cid // 2, cid % 2
        sl = slice(hg * DH, (hg + 1) * DH)
        in_maps.append({
            "xq": np.ascontiguousarray(q[b]),
            "xk": np.ascontiguousarray(k[b]),
            "xv": np.ascontiguousarray(v[b]),
            "wqt": np.ascontiguousarray(Wq[sl, :].T),
            "bq": np.ascontiguousarray(bq[sl]),
            "wot": np.ascontiguousarray(Wo[:, sl].T),
        })
    try:
        res = run_bass_kernel_spmd(
            nc, in_maps, core_ids=list(range(8)), trace=_trace
        )
    except ModuleNotFoundError:
        # NTFF profiling hook unavailable (axon client without antenv
        # profiling support) -- rerun without tracing
        res = run_bass_kernel_spmd(
            nc, in_maps, core_ids=list(range(8)), trace=False
        )
    parts = [r["out"] for r in res.results]
    outv = np.stack([parts[2 * b] + parts[2 * b + 1] for b in range(B)])
    outv = outv + bo[None, None, :]
    if _trace:
        kernel.last_result = res
    return outv[None].astype(np.float32)
